# revision 40
# baseline (speedup 1.0000x reference)
"""Trainium2 Bass kernel for nn_ChebyUpsample.

Reference computes: nearest-neighbor 2x upsample of x[B, T], then an
anti-causal 8th-order Chebyshev-I lowpass (4 cascaded biquads applied to the
time-reversed signal), i.e. an LTI filter y[n] = sum_{k>=0} h[k] up[n+k]
with zero-padding past the end of the signal.

Since |h[k]| decays like 0.933^k (slowest pole radius), h is numerically
zero past ~300 taps.  We fold the 2x upsample into two polyphase FIR
filters on the original grid:

    y[2i+phi] = sum_j g_phi[j] x[i+j],   g_phi built from adjacent h taps.

On a core, each row x[32768] is viewed as X[a, c] = x[c*128 + a] (fine time
on partitions).  For output block c (128 outputs), taps 0..127-m come from
block c, taps 128-m..255-m from block c+1, giving two dense 128x128
Toeplitz weight matrices per phase.  Both phases are fused into one moving
operand of 256 columns whose columns are pre-interleaved so the matmul
writes the output in its natural interleaved order:

    Z[b, 2m+phi] = sum_a T[a, b] * W_A[a, 2m+phi] + T'[a, b] * W_B[a, 2m+phi]

Per row: 1 DMA in, 2 PE transposes (natural layout -> fine-time-on-
partitions), 4 matmuls, PSUM->SBUF copies, 1 DMA out of the fully
assembled contiguous output row.  Pure data parallelism: 32 rows per core,
8 cores.
"""

from contextlib import ExitStack

import numpy as np

import concourse.bass as bass
import concourse.mybir as mybir
import concourse.tile as tile
from concourse import bass_utils

N_CORES = 8
B, T = 256, 32768
ROWS = B // N_CORES  # rows per core
TOUT = 2 * T
NBLK = T // 128  # 256 fine-time blocks per row
HALF = NBLK // 2  # 128

# float32r = full-rate fp32 PE path (1 cyc/row when moving free dim >= 256,
# vs 4 cyc/row for plain fp32); numerics verified against the reference on HW
_MM_DT = mybir.dt.float32r


def _cheby1_sos(N, rp, Wn):
    eps = np.sqrt(10.0 ** (0.1 * rp) - 1.0)
    mu = np.arcsinh(1.0 / eps) / N
    m = np.arange(-N + 1, N, 2)
    theta = np.pi * m / (2.0 * N)
    p = -np.sinh(mu + 1j * theta)
    k = np.prod(-p).real
    if N % 2 == 0:
        k /= np.sqrt(1.0 + eps * eps)
    fs = 2.0
    warped = 2.0 * fs * np.tan(np.pi * Wn / fs)
    p = p * warped
    k = k * warped**N
    fs2 = 2.0 * fs
    pz = (fs2 + p) / (fs2 - p)
    kz = k * np.real(1.0 / np.prod(fs2 - p))
    pos = pz[np.imag(pz) > 0]
    sos = np.zeros((N // 2, 6), dtype=np.float64)
    for i, pp in enumerate(pos):
        sos[i, 0:3] = [1.0, 2.0, 1.0]
        sos[i, 3:6] = [1.0, -2.0 * pp.real, abs(pp) ** 2]
    sos[0, 0:3] *= kz
    return sos


def _weights():
    """Host-side: impulse response -> polyphase Toeplitz operands (fp64->fp32)."""
    sos = _cheby1_sos(8, 0.05, 0.5)
    L = 1024
    h = np.zeros(L)
    h[0] = 1.0
    for b0, b1, b2, _a0, a1, a2 in sos:
        y = np.zeros(L)
        z1 = z2 = 0.0
        for n in range(L):
            yn = b0 * h[n] + z1
            z1 = b1 * h[n] - a1 * yn + z2
            z2 = b2 * h[n] - a2 * yn
            y[n] = yn
        h = y
    # polyphase taps on the x grid: y[2i+phi] = sum_j g_phi[j] x[i+j]
    ng = 256
    g0 = np.zeros(ng)
    g1 = np.zeros(ng)
    for j in range(ng):
        g0[j] = h[2 * j] + h[2 * j + 1]
        g1[j] = (h[2 * j - 1] if j >= 1 else 0.0) + h[2 * j]
    a = np.arange(128)[:, None]
    m = np.arange(128)[None, :]

    def toep(g):
        A = np.where(a - m >= 0, g[np.clip(a - m, 0, ng - 1)], 0.0)
        Bm = g[np.clip(128 + a - m, 0, ng - 1)]  # 128+a-m in [1, 255]
        return A, Bm

    A0, B0 = toep(g0)
    A1, B1 = toep(g1)
    # interleave phases into the moving operand's columns: col 2m+phi
    W_A = np.zeros((128, 256))
    W_B = np.zeros((128, 256))
    W_A[:, 0::2] = A0
    W_A[:, 1::2] = A1
    W_B[:, 0::2] = B0
    W_B[:, 1::2] = B1
    consts = np.concatenate([W_A, W_B, np.eye(128)], axis=1)
    return np.ascontiguousarray(consts, dtype=np.float32)


def _build(
    in_batch=8,
    out_batch=4,
    xc_bufs=4,
    t01_bufs=6,
    z_bufs=8,
    tps_bufs=2,
    zp_bufs=3,
    ze_on_act=False,
    in_split=1,
    out_split=1,
    out_dma_on_act=False,
    stage_rows=1,
):
    nc = bass.Bass("TRN2", debug=False, num_devices=N_CORES)
    f32 = mybir.dt.float32
    x = nc.dram_tensor("x", [ROWS, T], f32, kind="ExternalInput").ap()
    # [W_A | W_B | ident] packed into one tensor -> one DMA, one wait
    consts = nc.dram_tensor("consts", [128, 640], f32, kind="ExternalInput").ap()
    y = nc.dram_tensor("y", [ROWS, TOUT], f32, kind="ExternalOutput").ap()

    # Note: every TRN2 engine instruction has a single embedded wait slot;
    # _split_multi_waits() below legalizes any instruction Tile schedules
    # with more waits.  The structure here (one consts DMA absorbed by a
    # dummy transpose, pad columns zeroed once at startup, single-engine
    # PSUM consumers) keeps most instructions at <=1 wait to begin with.
    with tile.TileContext(nc) as tc, ExitStack() as ctx:
        const = ctx.enter_context(tc.tile_pool(name="const", bufs=1))
        xc_pool = ctx.enter_context(tc.tile_pool(name="xc", bufs=xc_bufs))
        t_pool = ctx.enter_context(tc.tile_pool(name="tt", bufs=t01_bufs))
        z_pool = ctx.enter_context(tc.tile_pool(name="z", bufs=z_bufs))
        ps_t = ctx.enter_context(tc.tile_pool(name="ps_t", bufs=tps_bufs, space="PSUM"))
        ps_z = ctx.enter_context(tc.tile_pool(name="ps_z", bufs=zp_bufs, space="PSUM"))

        ct = const.tile([128, 640], f32)
        nc.sync.dma_start(ct[:], consts)
        id_t = ct[:, 512:640]
        if _MM_DT == f32:
            wa_t = ct[:, 0:256]
            wb_t = ct[:, 256:512]
        else:
            # fp32r consumers require a producer that rounds to fp32r
            ctr = const.tile([128, 512], _MM_DT)
            nc.vector.tensor_copy(ctr[:], ct[:, 0:512])
            wa_t = ctr[:, 0:256]
            wb_t = ctr[:, 256:512]

        # dummy PE op: absorbs the const-DMA wait so per-row PE instructions
        # never need it
        scratch = ps_t.tile([128, 256], f32, tag="tps")
        nc.tensor.transpose(scratch[:, 0:128], id_t, id_t)

        # zero the pad column(s) of every t01 slot once
        SR = stage_rows
        for _ in range(t01_bufs):
            tz = t_pool.tile([128, 257 * SR], f32, tag="t01")
            nc.vector.memset(
                tz.rearrange("p (r c) -> p r c", c=257)[:, :, 256:257], 0.0
            )

        # batch schedules: small first/last batches shorten pipeline ramp/tail
        if isinstance(in_batch, int):
            in_sched = [in_batch] * (ROWS // in_batch)
        else:
            in_sched = list(in_batch)
        if isinstance(out_batch, int):
            out_sched = [out_batch] * (ROWS // out_batch)
        else:
            out_sched = list(out_batch)
        assert sum(in_sched) == ROWS and sum(out_sched) == ROWS

        in_start = {}
        pos = 0
        for b in in_sched:
            for k in range(b):
                in_start[pos + k] = (pos, b, k)
            pos += b
        out_start = {}
        pos = 0
        for b in out_sched:
            for k in range(b):
                out_start[pos + k] = (pos, b, k)
            pos += b

        z2 = None
        for r in range(ROWS):
            _, ib, k = in_start[r]
            if k == 0:
                xc4 = xc_pool.tile([128, ib * 256], f32, tag="xc")
                hb = ib // in_split
                for s in range(in_split):
                    nc.sync.dma_start(
                        xc4[:, s * hb * 256 : (s + 1) * hb * 256],
                        x[r + s * hb : r + (s + 1) * hb].rearrange(
                            "r (p f) -> p r f", p=128
                        ),
                    )
            # fine-time-on-partitions, ordered [T1 | T0] per row so that the
            # one-block-shifted T0 slice can run into the zero pad column:
            #   T1 = odd blocks 2b+1, T0 = even blocks 2b
            # stage_rows rows share one PSUM tile + one staging copy
            sk = r % SR
            if sk == 0:
                tps = ps_t.tile([128, 256 * SR], f32, tag="tps")
                t01 = t_pool.tile([128, 257 * SR], _MM_DT, tag="t01")
                for s in range(SR):
                    xcs = xc4[:, (k + s) * 256 : (k + s + 1) * 256]
                    nc.tensor.transpose(
                        tps[:, s * 256 : s * 256 + 128], xcs[:, 128:256], id_t
                    )
                    nc.tensor.transpose(
                        tps[:, s * 256 + 128 : s * 256 + 256], xcs[:, 0:128], id_t
                    )
                # single [T1 | T0 | 0]* staging copy; doubles as the
                # fp32 -> fp32r rounding step
                nc.vector.tensor_copy(
                    t01.rearrange("p (r c) -> p r c", c=257)[:, :, 0:256],
                    tps.rearrange("p (r c) -> p r c", c=256),
                )
            t1s = t01[:, sk * 257 : sk * 257 + 128]
            t0s = t01[:, sk * 257 + 128 : sk * 257 + 256]
            t0shift = t01[:, sk * 257 + 129 : sk * 257 + 257]

            zpe = ps_z.tile([128, 256], f32, tag="zpe")
            zpo = ps_z.tile([128, 256], f32, tag="zpo")
            nc.tensor.matmul(zpe[:], t0s, wa_t, start=True, stop=False)
            nc.tensor.matmul(zpe[:], t1s, wb_t, start=False, stop=True)
            nc.tensor.matmul(zpo[:], t1s, wa_t, start=True, stop=False)
            nc.tensor.matmul(zpo[:], t0shift, wb_t, start=False, stop=True)

            # PSUM -> SBUF copies split across DVE and ACT to balance load
            ostart, ob, j = out_start[r]
            if j == 0:
                z2 = z_pool.tile([128, ob * 512], f32, tag="z")
            zs = j * 512
            if ze_on_act:
                nc.scalar.copy(z2[:, zs : zs + 256], zpe[:])
            else:
                nc.vector.tensor_copy(z2[:, zs : zs + 128], zpe[:, 0:128])
                nc.scalar.copy(z2[:, zs + 128 : zs + 256], zpe[:, 128:256])
            nc.scalar.copy(z2[:, zs + 256 : zs + 512], zpo[:])
            hb = ob // out_split
            if (j + 1) % hb == 0:
                s = j // hb
                out_eng = nc.scalar if out_dma_on_act else nc.sync
                out_eng.dma_start(
                    y[ostart + s * hb : ostart + (s + 1) * hb].rearrange(
                        "r (b q) -> b r q", b=128
                    ),
                    z2[:, s * hb * 512 : (s + 1) * hb * 512],
                )

    _split_multi_waits(nc)
    return nc


# Engine data instructions on TRN2 have a single embedded wait slot; this
# walrus build rejects instructions carrying more.  Hoist extra waits into
# standalone EventSemaphore (sequencer wait) instructions placed immediately
# before the instruction on the same engine queue — semantically identical.
_NO_HOIST = {"EventSemaphore", "Call"}


def _split_multi_waits(nc, max_embedded=1):
    n_new = 0
    for f in nc.m.functions:
        for bb in f.blocks:
            il = bb.instructions
            new_list = []
            changed = False
            for inst in il:
                si = getattr(inst, "sync_info", None)
                opcode = str(getattr(inst, "opcode", ""))
                if si is not None and opcode not in _NO_HOIST:
                    waits = list(si.on_wait)
                    if len(waits) > max_embedded:
                        hoist, keep = waits[:-max_embedded], waits[-max_embedded:]
                        for w in hoist:
                            n_new += 1
                            new_list.append(
                                mybir.InstEventSemaphore(
                                    name=f"{inst.name}-hw{n_new}",
                                    ins=[],
                                    outs=[],
                                    sync_info=mybir.SyncInfo(
                                        on_wait=[w], on_update=[]
                                    ),
                                    engine=inst.engine,
                                )
                            )
                        inst.sync_info = mybir.SyncInfo(
                            on_wait=keep, on_update=list(si.on_update)
                        )
                        changed = True
                new_list.append(inst)
            if changed:
                bb.instructions = new_list
    return n_new


_CACHE = {}


def _get_nc():
    if "nc" not in _CACHE:
        _CACHE["nc"] = _build()
    return _CACHE["nc"]


def kernel(x, _trace=False, _trace_cores=None):
    x = np.ascontiguousarray(np.asarray(x), dtype=np.float32)
    assert x.shape == (B, T), x.shape
    consts = _weights()
    nc = _get_nc()
    in_maps = [
        {
            "x": np.ascontiguousarray(x[c * ROWS : (c + 1) * ROWS]),
            "consts": consts,
        }
        for c in range(N_CORES)
    ]
    res = bass_utils.run_bass_kernel_spmd(
        nc,
        in_maps,
        core_ids=list(range(N_CORES)),
        trace=_trace,
        trace_cores=_trace_cores,
    )
    _CACHE["last_results"] = res
    out = np.concatenate([r["y"] for r in res.results], axis=0)
    return out


# revision 44
# speedup vs baseline: 1.0038x; 1.0038x over previous
"""Trainium2 Bass kernel for nn_ChebyUpsample.

Reference computes: nearest-neighbor 2x upsample of x[B, T], then an
anti-causal 8th-order Chebyshev-I lowpass (4 cascaded biquads applied to the
time-reversed signal), i.e. an LTI filter y[n] = sum_{k>=0} h[k] up[n+k]
with zero-padding past the end of the signal.

Since |h[k]| decays like 0.933^k (slowest pole radius), h is numerically
zero past ~300 taps.  We fold the 2x upsample into two polyphase FIR
filters on the original grid:

    y[2i+phi] = sum_j g_phi[j] x[i+j],   g_phi built from adjacent h taps.

On a core, each row x[32768] is viewed as X[a, c] = x[c*128 + a] (fine time
on partitions).  For output block c (128 outputs), taps 0..127-m come from
block c, taps 128-m..255-m from block c+1, giving two dense 128x128
Toeplitz weight matrices per phase.  Both phases are fused into one moving
operand of 256 columns whose columns are pre-interleaved so the matmul
writes the output in its natural interleaved order:

    Z[b, 2m+phi] = sum_a T[a, b] * W_A[a, 2m+phi] + T'[a, b] * W_B[a, 2m+phi]

Per row: 1 DMA in, 2 PE transposes (natural layout -> fine-time-on-
partitions), 4 matmuls, PSUM->SBUF copies, 1 DMA out of the fully
assembled contiguous output row.  Pure data parallelism: 32 rows per core,
8 cores.
"""

from contextlib import ExitStack

import numpy as np

import concourse.bass as bass
import concourse.mybir as mybir
import concourse.tile as tile
from concourse import bass_utils

N_CORES = 8
B, T = 256, 32768
ROWS = B // N_CORES  # rows per core
TOUT = 2 * T
NBLK = T // 128  # 256 fine-time blocks per row
HALF = NBLK // 2  # 128

# float32r = full-rate fp32 PE path (1 cyc/row when moving free dim >= 256,
# vs 4 cyc/row for plain fp32); numerics verified against the reference on HW
_MM_DT = mybir.dt.float32r


def _cheby1_sos(N, rp, Wn):
    eps = np.sqrt(10.0 ** (0.1 * rp) - 1.0)
    mu = np.arcsinh(1.0 / eps) / N
    m = np.arange(-N + 1, N, 2)
    theta = np.pi * m / (2.0 * N)
    p = -np.sinh(mu + 1j * theta)
    k = np.prod(-p).real
    if N % 2 == 0:
        k /= np.sqrt(1.0 + eps * eps)
    fs = 2.0
    warped = 2.0 * fs * np.tan(np.pi * Wn / fs)
    p = p * warped
    k = k * warped**N
    fs2 = 2.0 * fs
    pz = (fs2 + p) / (fs2 - p)
    kz = k * np.real(1.0 / np.prod(fs2 - p))
    pos = pz[np.imag(pz) > 0]
    sos = np.zeros((N // 2, 6), dtype=np.float64)
    for i, pp in enumerate(pos):
        sos[i, 0:3] = [1.0, 2.0, 1.0]
        sos[i, 3:6] = [1.0, -2.0 * pp.real, abs(pp) ** 2]
    sos[0, 0:3] *= kz
    return sos


def _weights():
    """Host-side: impulse response -> polyphase Toeplitz operands (fp64->fp32)."""
    sos = _cheby1_sos(8, 0.05, 0.5)
    L = 1024
    h = np.zeros(L)
    h[0] = 1.0
    for b0, b1, b2, _a0, a1, a2 in sos:
        y = np.zeros(L)
        z1 = z2 = 0.0
        for n in range(L):
            yn = b0 * h[n] + z1
            z1 = b1 * h[n] - a1 * yn + z2
            z2 = b2 * h[n] - a2 * yn
            y[n] = yn
        h = y
    # polyphase taps on the x grid: y[2i+phi] = sum_j g_phi[j] x[i+j]
    ng = 256
    g0 = np.zeros(ng)
    g1 = np.zeros(ng)
    for j in range(ng):
        g0[j] = h[2 * j] + h[2 * j + 1]
        g1[j] = (h[2 * j - 1] if j >= 1 else 0.0) + h[2 * j]
    a = np.arange(128)[:, None]
    m = np.arange(128)[None, :]

    def toep(g):
        A = np.where(a - m >= 0, g[np.clip(a - m, 0, ng - 1)], 0.0)
        Bm = g[np.clip(128 + a - m, 0, ng - 1)]  # 128+a-m in [1, 255]
        return A, Bm

    A0, B0 = toep(g0)
    A1, B1 = toep(g1)
    # interleave phases into the moving operand's columns: col 2m+phi
    W_A = np.zeros((128, 256))
    W_B = np.zeros((128, 256))
    W_A[:, 0::2] = A0
    W_A[:, 1::2] = A1
    W_B[:, 0::2] = B0
    W_B[:, 1::2] = B1
    consts = np.concatenate([W_A, W_B, np.eye(128)], axis=1)
    return np.ascontiguousarray(consts, dtype=np.float32)


def _build(
    in_batch=8,
    out_batch=4,
    xc_bufs=4,
    t01_bufs=6,
    z_bufs=8,
    tps_bufs=2,
    zp_bufs=3,
    ze_on_act=False,
    in_split=1,
    out_split=1,
    out_dma_on_act=False,
    stage_rows=1,
    in_dma_on_act=False,
    tail_split=4,
    head_split=1,
):
    nc = bass.Bass("TRN2", debug=False, num_devices=N_CORES)
    f32 = mybir.dt.float32
    x = nc.dram_tensor("x", [ROWS, T], f32, kind="ExternalInput").ap()
    # [W_A | W_B | ident] packed into one tensor -> one DMA, one wait
    consts = nc.dram_tensor("consts", [128, 640], f32, kind="ExternalInput").ap()
    y = nc.dram_tensor("y", [ROWS, TOUT], f32, kind="ExternalOutput").ap()

    # Note: every TRN2 engine instruction has a single embedded wait slot;
    # _split_multi_waits() below legalizes any instruction Tile schedules
    # with more waits.  The structure here (one consts DMA absorbed by a
    # dummy transpose, pad columns zeroed once at startup, single-engine
    # PSUM consumers) keeps most instructions at <=1 wait to begin with.
    with tile.TileContext(nc) as tc, ExitStack() as ctx:
        const = ctx.enter_context(tc.tile_pool(name="const", bufs=1))
        xc_pool = ctx.enter_context(tc.tile_pool(name="xc", bufs=xc_bufs))
        t_pool = ctx.enter_context(tc.tile_pool(name="tt", bufs=t01_bufs))
        z_pool = ctx.enter_context(tc.tile_pool(name="z", bufs=z_bufs))
        ps_t = ctx.enter_context(tc.tile_pool(name="ps_t", bufs=tps_bufs, space="PSUM"))
        ps_z = ctx.enter_context(tc.tile_pool(name="ps_z", bufs=zp_bufs, space="PSUM"))

        ct = const.tile([128, 640], f32)
        nc.sync.dma_start(ct[:], consts)
        id_t = ct[:, 512:640]
        if _MM_DT == f32:
            wa_t = ct[:, 0:256]
            wb_t = ct[:, 256:512]
        else:
            # fp32r consumers require a producer that rounds to fp32r
            ctr = const.tile([128, 512], _MM_DT)
            nc.vector.tensor_copy(ctr[:], ct[:, 0:512])
            wa_t = ctr[:, 0:256]
            wb_t = ctr[:, 256:512]

        # dummy PE op: absorbs the const-DMA wait so per-row PE instructions
        # never need it
        scratch = ps_t.tile([128, 256], f32, tag="tps")
        nc.tensor.transpose(scratch[:, 0:128], id_t, id_t)

        # zero the pad column(s) of every t01 slot once
        SR = stage_rows
        for _ in range(t01_bufs):
            tz = t_pool.tile([128, 257 * SR], f32, tag="t01")
            nc.vector.memset(
                tz.rearrange("p (r c) -> p r c", c=257)[:, :, 256:257], 0.0
            )

        # batch schedules: small first/last batches shorten pipeline ramp/tail
        if isinstance(in_batch, int):
            in_sched = [in_batch] * (ROWS // in_batch)
        else:
            in_sched = list(in_batch)
        if isinstance(out_batch, int):
            out_sched = [out_batch] * (ROWS // out_batch)
        else:
            out_sched = list(out_batch)
        assert sum(in_sched) == ROWS and sum(out_sched) == ROWS

        in_start = {}
        pos = 0
        for b in in_sched:
            for k in range(b):
                in_start[pos + k] = (pos, b, k)
            pos += b
        out_start = {}
        pos = 0
        for b in out_sched:
            for k in range(b):
                out_start[pos + k] = (pos, b, k)
            pos += b

        z2 = None
        for r in range(ROWS):
            _, ib, k = in_start[r]
            if k == 0:
                xc4 = xc_pool.tile([128, ib * 256], f32, tag="xc")
                isplit = head_split if r == 0 else in_split
                in_eng = nc.scalar if in_dma_on_act else nc.sync
                hb = ib // isplit
                for s in range(isplit):
                    in_eng.dma_start(
                        xc4[:, s * hb * 256 : (s + 1) * hb * 256],
                        x[r + s * hb : r + (s + 1) * hb].rearrange(
                            "r (p f) -> p r f", p=128
                        ),
                    )
            # fine-time-on-partitions, ordered [T1 | T0] per row so that the
            # one-block-shifted T0 slice can run into the zero pad column:
            #   T1 = odd blocks 2b+1, T0 = even blocks 2b
            # stage_rows rows share one PSUM tile + one staging copy
            sk = r % SR
            if sk == 0:
                tps = ps_t.tile([128, 256 * SR], f32, tag="tps")
                t01 = t_pool.tile([128, 257 * SR], _MM_DT, tag="t01")
                for s in range(SR):
                    xcs = xc4[:, (k + s) * 256 : (k + s + 1) * 256]
                    nc.tensor.transpose(
                        tps[:, s * 256 : s * 256 + 128], xcs[:, 128:256], id_t
                    )
                    nc.tensor.transpose(
                        tps[:, s * 256 + 128 : s * 256 + 256], xcs[:, 0:128], id_t
                    )
                # single [T1 | T0 | 0]* staging copy; doubles as the
                # fp32 -> fp32r rounding step
                nc.vector.tensor_copy(
                    t01.rearrange("p (r c) -> p r c", c=257)[:, :, 0:256],
                    tps.rearrange("p (r c) -> p r c", c=256),
                )
            t1s = t01[:, sk * 257 : sk * 257 + 128]
            t0s = t01[:, sk * 257 + 128 : sk * 257 + 256]
            t0shift = t01[:, sk * 257 + 129 : sk * 257 + 257]

            zpe = ps_z.tile([128, 256], f32, tag="zpe")
            zpo = ps_z.tile([128, 256], f32, tag="zpo")
            nc.tensor.matmul(zpe[:], t0s, wa_t, start=True, stop=False)
            nc.tensor.matmul(zpe[:], t1s, wb_t, start=False, stop=True)
            nc.tensor.matmul(zpo[:], t1s, wa_t, start=True, stop=False)
            nc.tensor.matmul(zpo[:], t0shift, wb_t, start=False, stop=True)

            # PSUM -> SBUF copies split across DVE and ACT to balance load
            ostart, ob, j = out_start[r]
            if j == 0:
                z2 = z_pool.tile([128, ob * 512], f32, tag="z")
            zs = j * 512
            if ze_on_act:
                nc.scalar.copy(z2[:, zs : zs + 256], zpe[:])
            else:
                nc.vector.tensor_copy(z2[:, zs : zs + 128], zpe[:, 0:128])
                nc.scalar.copy(z2[:, zs + 128 : zs + 256], zpe[:, 128:256])
            nc.scalar.copy(z2[:, zs + 256 : zs + 512], zpo[:])
            osplit = out_split if ostart + ob < ROWS else max(out_split, tail_split)
            hb = ob // osplit
            if (j + 1) % hb == 0:
                s = j // hb
                out_eng = nc.scalar if out_dma_on_act else nc.sync
                out_eng.dma_start(
                    y[ostart + s * hb : ostart + (s + 1) * hb].rearrange(
                        "r (b q) -> b r q", b=128
                    ),
                    z2[:, s * hb * 512 : (s + 1) * hb * 512],
                )

    _split_multi_waits(nc)
    return nc


# Engine data instructions on TRN2 have a single embedded wait slot; this
# walrus build rejects instructions carrying more.  Hoist extra waits into
# standalone EventSemaphore (sequencer wait) instructions placed immediately
# before the instruction on the same engine queue — semantically identical.
_NO_HOIST = {"EventSemaphore", "Call"}


def _split_multi_waits(nc, max_embedded=1):
    n_new = 0
    for f in nc.m.functions:
        for bb in f.blocks:
            il = bb.instructions
            new_list = []
            changed = False
            for inst in il:
                si = getattr(inst, "sync_info", None)
                opcode = str(getattr(inst, "opcode", ""))
                if si is not None and opcode not in _NO_HOIST:
                    waits = list(si.on_wait)
                    if len(waits) > max_embedded:
                        hoist, keep = waits[:-max_embedded], waits[-max_embedded:]
                        for w in hoist:
                            n_new += 1
                            new_list.append(
                                mybir.InstEventSemaphore(
                                    name=f"{inst.name}-hw{n_new}",
                                    ins=[],
                                    outs=[],
                                    sync_info=mybir.SyncInfo(
                                        on_wait=[w], on_update=[]
                                    ),
                                    engine=inst.engine,
                                )
                            )
                        inst.sync_info = mybir.SyncInfo(
                            on_wait=keep, on_update=list(si.on_update)
                        )
                        changed = True
                new_list.append(inst)
            if changed:
                bb.instructions = new_list
    return n_new


_CACHE = {}


def _get_nc():
    if "nc" not in _CACHE:
        _CACHE["nc"] = _build()
    return _CACHE["nc"]


def kernel(x, _trace=False, _trace_cores=None):
    x = np.ascontiguousarray(np.asarray(x), dtype=np.float32)
    assert x.shape == (B, T), x.shape
    consts = _weights()
    nc = _get_nc()
    in_maps = [
        {
            "x": np.ascontiguousarray(x[c * ROWS : (c + 1) * ROWS]),
            "consts": consts,
        }
        for c in range(N_CORES)
    ]
    res = bass_utils.run_bass_kernel_spmd(
        nc,
        in_maps,
        core_ids=list(range(N_CORES)),
        trace=_trace,
        trace_cores=_trace_cores,
    )
    _CACHE["last_results"] = res
    out = np.concatenate([r["y"] for r in res.results], axis=0)
    return out
